# revision 19
# baseline (speedup 1.0000x reference)
"""Trainium2 Bass kernel for the conditioned WaveNet denoiser.

Distribution strategy (8 NeuronCores):
  - Data-parallel over batch: core b owns sample b end-to-end (block loop +
    output head), with the small weights replicated.
  - The huge stacked conditioning Dense weights Dt/Ds ([10,16,2048,128] f32,
    ~335 MB for the pair) are channel-sharded 8 ways.  Core j computes the
    conditioning planes trans[b, t, k] for ALL batches b over its 16 channels
    (a packed bf16 matmul against a host-built block-diagonal matrix of
    `condition`), then a chunked AllToAll routes each batch's planes to its
    owner core, overlapped with the residual-block compute.
  - All matmuls run at 1 cycle/row in bf16 (f32 PSUM accumulate; the
    f32 master copies of h/skip keep the residual chain precise).

kernel() accepts the FULL inputs and returns the FULL [8, 2048, 1] output.
"""

import os
import sys

import numpy as np

for _p in ("/opt/trn_rl_repo",):
    if _p not in sys.path and os.path.isdir(_p):
        sys.path.insert(0, _p)

import ml_dtypes  # noqa: E402

import concourse.bass as bass  # noqa: E402
import concourse.tile as tile  # noqa: E402
from concourse import bacc, bass_utils, mybir  # noqa: E402

# Problem constants (hardcoded per the spec; kernel.py must be self-contained).
L = 10
DILATIONS = [1, 2, 4, 8, 16, 32, 64, 128, 256, 512]
T = 2048
C = 128
COND = 16
B = 8
NCORES = 8
TS = 512          # time-tile (matmul moving N / one PSUM bank of f32)
NT = T // TS      # 4 time tiles
CHUNK = 2         # residual blocks per AllToAll chunk
NCHUNK = L // CHUNK
PLANES_PER_CHUNK = 2 * CHUNK  # (l, branch) planes per chunk

F32 = mybir.dt.float32
F32R = mybir.dt.float32r
BF16 = mybir.dt.bfloat16
BF = ml_dtypes.bfloat16

AF = mybir.ActivationFunctionType


def _r(ap):
    """View an f32 AP as float32r for full-rate PE matmuls."""
    return ap.bitcast(F32R)


def _tap_range(t0, n, off):
    """Valid (out_lo, length) of an out tile [t0, t0+n) for input offset off."""
    lo = max(t0, -off)
    hi = min(t0 + n, T - off)
    return lo - t0, max(0, hi - lo)


def _build_nc(has_p: bool, has_bres: bool, has_bskip: bool):
    nc = bacc.Bacc(
        "TRN2",
        target_bir_lowering=False,
        debug=False,
        num_devices=NCORES,
    )

    # ---- I/O declarations (per-core values supplied via in_maps) ----
    xw = nc.dram_tensor("xw", [1, T], F32, kind="ExternalInput")
    wcT = nc.dram_tensor("wcT", [1, C], F32, kind="ExternalInput")
    bcp = nc.dram_tensor("bcp", [C, 1], F32, kind="ExternalInput")
    cstat = nc.dram_tensor("cstat", [C, 64], BF16, kind="ExternalInput")
    # [lb, p, hh, t] so one plane-pair is a single contiguous-line DMA
    dtp = nc.dram_tensor("dtp", [2 * L, C, 2, T], BF16, kind="ExternalInput")
    wtp = nc.dram_tensor("wtp", [C, 6 * L, C], BF16, kind="ExternalInput")
    wsr = nc.dram_tensor("wsr", [C, 2 * L, C], BF16, kind="ExternalInput")
    w1p = nc.dram_tensor("w1p", [C, 3, 2048], BF16, kind="ExternalInput")
    b1p = nc.dram_tensor("b1p", [C, 16], F32, kind="ExternalInput")
    w2p = nc.dram_tensor("w2p", [C, 96, C], BF16, kind="ExternalInput")
    b2p = nc.dram_tensor("b2p", [C, 2], F32, kind="ExternalInput")
    w3p = nc.dram_tensor("w3p", [C, 2], BF16, kind="ExternalInput")
    b3p = nc.dram_tensor("b3p", [1, 1], F32, kind="ExternalInput")
    ident = nc.dram_tensor("ident", [C, C], BF16, kind="ExternalInput")
    if has_p:
        ptp = nc.dram_tensor("ptp", [2 * L, 16, T], BF16, kind="ExternalInput")
        pstat = nc.dram_tensor("pstat", [8, 64], BF16, kind="ExternalInput")
    if has_bres:
        bresp = nc.dram_tensor("bresp", [C, L], F32, kind="ExternalInput")
    if has_bskip:
        bskips = nc.dram_tensor("bskips", [C, 1], F32, kind="ExternalInput")
    out = nc.dram_tensor("out", [1, T], F32, kind="ExternalOutput")

    rg = [list(range(NCORES))]

    with tile.TileContext(nc) as tc:
        with (
            tc.tile_pool(name="consts", bufs=1) as consts,
            tc.tile_pool(name="skipbuf", bufs=1) as skipbuf,
            tc.tile_pool(name="dram", bufs=1, space="DRAM") as dram,
        ):
            # ---- small constants (hot path first) ----
            x_sb = consts.tile([1, T], F32)
            nc.sync.dma_start(x_sb[:], xw[:, :])
            wc_sb = consts.tile([1, C], F32)
            nc.sync.dma_start(wc_sb[:], wcT[:, :])
            bc_sb = consts.tile([C, 1], F32)
            nc.sync.dma_start(bc_sb[:], bcp[:, :])
            cs_sb = consts.tile([C, 64], BF16)
            nc.sync.dma_start(cs_sb[:], cstat[:, :])
            id_sb = consts.tile([C, C], BF16)
            nc.sync.dma_start(id_sb[:], ident[:, :])
            wt_sb = consts.tile([C, 6 * L, C], BF16)
            nc.gpsimd.dma_start(wt_sb[:], wtp[:, :, :])
            wsr_sb = consts.tile([C, 2 * L, C], BF16)
            nc.gpsimd.dma_start(wsr_sb[:], wsr[:, :, :])
            if has_p:
                ps_sb = consts.tile([8, 64], BF16)
                nc.sync.dma_start(ps_sb[:], pstat[:, :])
            if has_bres:
                bres_sb = consts.tile([C, L], F32)
                nc.sync.dma_start(bres_sb[:], bresp[:, :])
            if has_bskip:
                bsk_sb = consts.tile([C, 1], F32)
                nc.sync.dma_start(bsk_sb[:], bskips[:, :])

            # AllToAll bounce buffers, one pair per chunk.
            a2a_in = []
            a2a_out = []
            for c in range(NCHUNK):
                ain = dram.tile(
                    [B, PLANES_PER_CHUNK, 16, T], BF16, name=f"a2a_in{c}"
                )
                aout = dram.tile(
                    [B, PLANES_PER_CHUNK, 16, T], BF16, name=f"a2a_out{c}"
                )
                a2a_in.append(ain)
                a2a_out.append(aout)

            # All producer + consumer pools share one scope so PSUM banks and
            # SBUF regions never alias between the two concurrent phases
            # (aliasing adds release-deps that serialize them).
            with (
                tc.tile_pool(name="hbuf", bufs=2) as hbuf,
                tc.tile_pool(name="hbfbuf", bufs=2) as hbfbuf,
                tc.tile_pool(name="dtbuf", bufs=3) as dtbuf,
                tc.tile_pool(name="ptbuf", bufs=2) as ptbuf,
                tc.tile_pool(name="stgbuf", bufs=4) as stgbuf,
                tc.tile_pool(name="ttbuf", bufs=6) as ttbuf,
                tc.tile_pool(name="gbuf", bufs=2) as gbuf,
                tc.tile_pool(name="gtmp", bufs=4) as gtmp,
                tc.tile_pool(name="psum_prod", bufs=3, space="PSUM") as psum_prod,
                tc.tile_pool(name="psum_z", bufs=2, space="PSUM") as psum_z,
                tc.tile_pool(name="psum_sr", bufs=3, space="PSUM") as psum_sr,
            ):
                # ---- h = x * Wc + bc  (K=1 f32 matmul + biased copy) ----
                h = hbuf.tile([C, T], F32, name="h0")
                h_bf = hbfbuf.tile([C, T], BF16, name="hbf0")
                for it in range(NT):
                    ph = psum_z.tile([C, TS], F32, name="ph", tag="z")
                    nc.tensor.matmul(
                        ph[:],
                        wc_sb[:, :],
                        x_sb[:, bass.ts(it, TS)],
                        start=True,
                        stop=True,
                    )
                    nc.scalar.activation(
                        h[:, bass.ts(it, TS)], ph[:], AF.Identity, bias=bc_sb[:, 0:1]
                    )
                    nc.vector.tensor_copy(
                        h_bf[:, bass.ts(it, TS)], h[:, bass.ts(it, TS)]
                    )

                # ---- producer: conditioning planes + chunked AllToAll ----
                for cki in range(NCHUNK):
                    for lbc in range(PLANES_PER_CHUNK):
                        lb = cki * PLANES_PER_CHUNK + lbc
                        dt2 = dtbuf.tile([C, 2, T], BF16, name="dt2")
                        nc.scalar.dma_start(dt2[:], dtp[lb])
                        if has_p:
                            pt = ptbuf.tile([16, T], BF16, name="pt")
                            nc.sync.dma_start(pt[:], ptp[lb])
                        stg = stgbuf.tile([C, T], BF16, name="stg")
                        for it in range(NT):
                            ppr = psum_prod.tile([C, TS], F32, name="ppr")
                            tsl = bass.ts(it, TS)
                            for hh in range(2):
                                rows = slice(64 * hh, 64 * hh + 64)
                                last_prod_mm = nc.tensor.matmul(
                                    ppr[rows, :],
                                    cs_sb[:, :],
                                    dt2[:, hh, tsl],
                                    start=True,
                                    stop=not has_p,
                                )
                                if has_p:
                                    nc.tensor.matmul(
                                        ppr[rows, :],
                                        ps_sb[:, :],
                                        pt[8 * hh : 8 * hh + 8, tsl],
                                        start=False,
                                        stop=True,
                                    )
                            nc.vector.tensor_copy(stg[:, tsl], ppr[:])
                        for hh in range(2):
                            nc.sync.dma_start(
                                a2a_in[cki][:, lbc, 8 * hh : 8 * hh + 8, :],
                                stg[64 * hh : 64 * hh + 64, :],
                            )
                    nc.gpsimd.collective_compute(
                        "AllToAll",
                        mybir.AluOpType.bypass,
                        replica_groups=rg,
                        ins=[a2a_in[cki][:, :, :, :].opt()],
                        outs=[a2a_out[cki][:, :, :, :].opt()],
                    )

                # ---- residual block loop (consumer) ----
                # Keep the in-order PE stream strictly producer-first: a
                # consumer matmul scheduled between producer matmuls would
                # head-of-line-block them while waiting for its AllToAll.
                prod_fence = last_prod_mm
                skip_sb = skipbuf.tile([C, T], F32, name="skip")
                for l in range(L):
                    d = DILATIONS[l]
                    cki, lrel = divmod(l, CHUNK)
                    planes = []
                    for br in range(2):
                        tb = ttbuf.tile([C, T], BF16, name="tb")
                        nc.gpsimd.dma_start(
                            tb[:], a2a_out[cki][:, 2 * lrel + br, :, :]
                        )
                        planes.append(tb)
                    g = gbuf.tile([C, T], BF16, name="g")
                    h_new = hbuf.tile([C, T], F32, name="hn")
                    h_bf_new = hbfbuf.tile([C, T], BF16, name="hbn")
                    for it in range(NT):
                        t0 = it * TS
                        tsl = bass.ts(it, TS)
                        acts = []
                        for br, fn in ((0, AF.Tanh), (1, AF.Sigmoid)):
                            pz = psum_z.tile([C, TS], F32, name="pz", tag="z")
                            taps = []
                            for tap, off in ((1, 0), (0, -d), (2, d)):
                                lo, n = _tap_range(t0, TS, off)
                                if n > 0:
                                    taps.append((tap, off, lo, n))
                            for idx, (tap, off, lo, n) in enumerate(taps):
                                w_ap = wt_sb[:, (l * 2 + br) * 3 + tap, :]
                                mm = nc.tensor.matmul(
                                    pz[:, lo : lo + n],
                                    w_ap,
                                    h_bf[:, t0 + lo + off : t0 + lo + off + n],
                                    start=idx == 0,
                                    stop=idx == len(taps) - 1,
                                )
                                if prod_fence is not None:
                                    tile.add_dep_helper(
                                        mm.ins,
                                        prod_fence.ins,
                                        reason="consumer after producer",
                                    )
                                    prod_fence = None
                            zs = gtmp.tile([C, TS], F32, name="zs", tag="zs")
                            nc.vector.tensor_add(
                                zs[:], pz[:], planes[br][:, tsl]
                            )
                            av = gtmp.tile([C, TS], F32, name="av", tag="av")
                            nc.scalar.activation(av[:], zs[:], fn)
                            acts.append(av)
                        nc.vector.tensor_mul(
                            g[:, tsl], acts[0][:], acts[1][:]
                        )
                        # skip 1x1 conv, accumulated in SBUF
                        psk = psum_sr.tile([C, TS], F32, name="psk", tag="sr")
                        nc.tensor.matmul(
                            psk[:],
                            wsr_sb[:, 2 * l, :],
                            g[:, tsl],
                            start=True,
                            stop=True,
                        )
                        if l == 0:
                            nc.vector.tensor_copy(skip_sb[:, tsl], psk[:])
                        else:
                            nc.vector.tensor_add(
                                skip_sb[:, tsl], skip_sb[:, tsl], psk[:]
                            )
                        # residual 1x1 conv + h
                        prs = psum_sr.tile([C, TS], F32, name="prs", tag="sr")
                        nc.tensor.matmul(
                            prs[:],
                            wsr_sb[:, 2 * l + 1, :],
                            g[:, tsl],
                            start=True,
                            stop=True,
                        )
                        nc.vector.tensor_add(h_new[:, tsl], prs[:], h[:, tsl])
                        if has_bres:
                            nc.scalar.activation(
                                h_new[:, tsl],
                                h_new[:, tsl],
                                AF.Identity,
                                bias=bres_sb[:, l : l + 1],
                            )
                        nc.vector.tensor_copy(h_bf_new[:, tsl], h_new[:, tsl])
                    h = h_new
                    h_bf = h_bf_new

                if has_bskip:
                    nc.scalar.activation(
                        skip_sb[:, :], skip_sb[:, :], AF.Identity, bias=bsk_sb[:, 0:1]
                    )
                skip_bf = skipbuf.tile([C, T], BF16, name="skipbf")
                for it in range(NT):
                    nc.vector.tensor_copy(
                        skip_bf[:, bass.ts(it, TS)], skip_sb[:, bass.ts(it, TS)]
                    )

            # ---- output head ----
            with (
                tc.tile_pool(name="o1buf", bufs=1) as o1buf,
                tc.tile_pool(name="o2buf", bufs=1) as o2buf,
                tc.tile_pool(name="obuf", bufs=1) as obuf,
                tc.tile_pool(name="psum_h1", bufs=2, space="PSUM") as psum_h1,
                tc.tile_pool(name="psum_h2", bufs=2, space="PSUM") as psum_h2,
                tc.tile_pool(name="psum_h3", bufs=2, space="PSUM") as psum_h3,
                tc.tile_pool(name="headw", bufs=1) as headw,
            ):
                w1_sb = headw.tile([C, 3, 2048], BF16)
                nc.sync.dma_start(w1_sb[:], w1p[:, :, :])
                b1_sb = headw.tile([C, 16], F32)
                nc.sync.dma_start(b1_sb[:], b1p[:, :])
                w2_sb = headw.tile([C, 96, C], BF16)
                nc.sync.dma_start(w2_sb[:], w2p[:, :, :])
                b2_sb = headw.tile([C, 2], F32)
                nc.sync.dma_start(b2_sb[:], b2p[:, :])
                w3_sb = headw.tile([C, 2], BF16)
                nc.sync.dma_start(w3_sb[:], w3p[:, :])
                b3_sb = headw.tile([1, 1], F32)
                nc.sync.dma_start(b3_sb[:], b3p[:, :])

                out1 = o1buf.tile([C, 16, T], BF16, name="out1")
                out2 = o2buf.tile([C, 2, T], BF16, name="out2")
                o_sb = obuf.tile([1, T], F32, name="o_sb")
                for oc in range(16):
                    for it in range(NT):
                        t0 = it * TS
                        p1 = psum_h1.tile([C, TS], F32, name="p1")
                        taps = []
                        for tap, off in ((1, 0), (0, -1), (2, 1)):
                            lo, n = _tap_range(t0, TS, off)
                            if n > 0:
                                taps.append((tap, off, lo, n))
                        for idx, (tap, off, lo, n) in enumerate(taps):
                            w_ap = w1_sb[:, tap, oc * C : (oc + 1) * C]
                            nc.tensor.matmul(
                                p1[:, lo : lo + n],
                                w_ap,
                                skip_bf[:, t0 + lo + off : t0 + lo + off + n],
                                start=idx == 0,
                                stop=idx == len(taps) - 1,
                            )
                        nc.scalar.activation(
                            out1[:, oc, bass.ts(it, TS)],
                            p1[:],
                            AF.Relu,
                            bias=b1_sb[:, oc : oc + 1],
                        )
                for oc2 in range(2):
                    for it in range(NT):
                        t0 = it * TS
                        p2 = psum_h2.tile([C, TS], F32, name="p2")
                        taps = []
                        for tap, off in ((1, 0), (0, -1), (2, 1)):
                            lo, n = _tap_range(t0, TS, off)
                            if n > 0:
                                taps.append((tap, off, lo, n))
                        nmm = len(taps) * 16
                        k = 0
                        for tap, off, lo, n in taps:
                            for ic in range(16):
                                w_ap = w2_sb[:, (tap * 16 + ic) * 2 + oc2, :]
                                nc.tensor.matmul(
                                    p2[:, lo : lo + n],
                                    w_ap,
                                    out1[:, ic, t0 + lo + off : t0 + lo + off + n],
                                    start=k == 0,
                                    stop=k == nmm - 1,
                                )
                                k += 1
                        nc.scalar.activation(
                            out2[:, oc2, bass.ts(it, TS)],
                            p2[:],
                            AF.Relu,
                            bias=b2_sb[:, oc2 : oc2 + 1],
                        )
                for it in range(NT):
                    tsl = bass.ts(it, TS)
                    p3 = psum_h3.tile([1, TS], F32, name="p3")
                    for ic in range(2):
                        nc.tensor.matmul(
                            p3[:],
                            w3_sb[:, ic : ic + 1],
                            out2[:, ic, tsl],
                            start=ic == 0,
                            stop=ic == 1,
                        )
                    nc.scalar.activation(
                        o_sb[:, tsl], p3[:], AF.Tanh, bias=b3_sb[:, 0:1]
                    )
                nc.sync.dma_start(out[:, :], o_sb[:])

    nc.compile()
    return nc


_NC_CACHE = {}


def _get_nc(has_p, has_bres, has_bskip):
    key = (has_p, has_bres, has_bskip)
    if key not in _NC_CACHE:
        _NC_CACHE[key] = _build_nc(*key)
    return _NC_CACHE[key]


def _pack_inputs(
    x, condition, Wc, bc, Wt, bt, Ws, bs, Dt, Bt, Ds, Bs,
    Wskip, bskip, Wres, bres, W1, b1, W2, b2, W3, b3,
):
    """Host-side sharding + layout packs. Returns (in_maps, flags)."""
    f32 = np.float32
    x = np.asarray(x, f32)
    condition = np.asarray(condition, f32)
    has_p = bool(
        np.any(np.asarray(Bt)) or np.any(np.asarray(Bs))
        or np.any(np.asarray(bt)) or np.any(np.asarray(bs))
    )
    has_bres = bool(np.any(np.asarray(bres)))
    has_bskip = bool(np.any(np.asarray(bskip)))

    # dtp: [core, lb=2l+br, hh, p=16g+c, t] = D_br[l, c, t, 16j+8hh+g]
    D = np.stack([np.asarray(Dt, f32), np.asarray(Ds, f32)], axis=1)
    D = D.reshape(L, 2, COND, T, 8, 2, 8)
    # [core, lb, p=16g+c, hh, t]
    dtp_all = np.ascontiguousarray(
        D.transpose(4, 0, 1, 6, 2, 5, 3).reshape(NCORES, 2 * L, C, 2, T)
    ).astype(BF)
    del D

    # cstat: [16g+c, 8b+g] = condition[b, c]
    cstat = np.zeros((C, 64), f32)
    for g in range(8):
        cstat[16 * g : 16 * g + 16, g::8] = condition.T
    cstat = cstat.astype(BF)

    # wtp: [cin, (l,br,tap), cout]
    Wg = np.stack([np.asarray(Wt, f32), np.asarray(Ws, f32)], axis=1)
    wtp = np.ascontiguousarray(
        Wg.transpose(3, 0, 1, 2, 4).reshape(C, 6 * L, C)
    ).astype(BF)
    # wsr: [cin, (l, skip/res), cout]
    Ssr = np.stack([np.asarray(Wskip, f32)[:, 0], np.asarray(Wres, f32)[:, 0]], axis=1)
    wsr = np.ascontiguousarray(Ssr.transpose(2, 0, 1, 3).reshape(C, 2 * L, C)).astype(BF)

    w1p = np.ascontiguousarray(np.asarray(W1, f32).transpose(1, 0, 2)).astype(BF)
    b1p = np.ascontiguousarray(np.asarray(b1, f32).reshape(16, C).T)
    w2p = np.ascontiguousarray(
        np.asarray(W2, f32).reshape(3, 16, C, 2, C).transpose(2, 0, 1, 3, 4)
        .reshape(C, 96, C)
    ).astype(BF)
    b2p = np.ascontiguousarray(np.asarray(b2, f32).reshape(2, C).T)
    w3p = np.ascontiguousarray(np.asarray(W3, f32)[0, :, 0].reshape(2, C).T).astype(BF)
    b3p = np.asarray(b3, f32).reshape(1, 1)
    wcT = np.ascontiguousarray(np.asarray(Wc, f32).reshape(1, C))
    bcp = np.asarray(bc, f32).reshape(C, 1)
    ident = np.eye(C, dtype=f32).astype(BF)

    base = {
        "wcT": wcT, "bcp": bcp, "cstat": cstat, "wtp": wtp, "wsr": wsr,
        "w1p": w1p, "b1p": b1p, "w2p": w2p, "b2p": b2p, "w3p": w3p,
        "b3p": b3p, "ident": ident,
    }
    if has_p:
        P = np.stack(
            [
                np.asarray(Bt, f32) + np.asarray(bt, f32)[:, None, :],
                np.asarray(Bs, f32) + np.asarray(bs, f32)[:, None, :],
            ],
            axis=1,
        )  # [L, 2, T, C]
        P = P.reshape(L, 2, T, 8, 2, 8)
        ptp_all = np.ascontiguousarray(
            P.transpose(3, 0, 1, 4, 5, 2).reshape(NCORES, 2 * L, 16, T)
        ).astype(BF)
        del P
        pstat = np.zeros((8, 64), f32)
        for g in range(8):
            pstat[g, g::8] = 1.0
        base["pstat"] = pstat.astype(BF)
    if has_bres:
        base["bresp"] = np.ascontiguousarray(np.asarray(bres, f32).T)
    if has_bskip:
        base["bskips"] = np.asarray(bskip, f32).sum(axis=0).reshape(C, 1)

    in_maps = []
    for j in range(NCORES):
        m = dict(base)
        m["xw"] = np.ascontiguousarray(x[j, :, 0].reshape(1, T))
        m["dtp"] = dtp_all[j]
        if has_p:
            m["ptp"] = ptp_all[j]
        in_maps.append(m)
    return in_maps, (has_p, has_bres, has_bskip)


def kernel(**inputs) -> np.ndarray:
    in_maps, flags = _pack_inputs(**inputs)
    nc = _get_nc(*flags)
    res = bass_utils.run_bass_kernel_spmd(
        nc, in_maps, core_ids=list(range(NCORES))
    )
    outs = [res.results[j]["out"].reshape(T, 1) for j in range(NCORES)]
    return np.stack(outs, axis=0).astype(np.float32)
